# revision 36
# baseline (speedup 1.0000x reference)
"""AnomalyAttention (two causal attentions per (b,h)) on 8 TRN2 NeuronCores.

Sharding: B*H = 16 (batch, head) pairs -> 2 pairs per core. Each core runs
4 independent causal attentions (time + channel for each of its 2 pairs).
No cross-core communication.

Per-attention layout ("transposed flash"): keys live on SBUF partitions.
  S^T[k, q]   = kT_tile.T @ qT                (PE bf16, contraction E=64;
                the two attn types use PE row-groups 0-63 / 64-127)
  P^T         = exp(scale * S^T)              (ACT, PSUM -> SBUF bf16)
  diag mask   = affine_select zero-triangle   (GPSIMD, on the diag block)
  outT[d, q] += V_ext[k, d].T @ P^T[k, q]     (PE bf16, PSUM-accumulated)
V_ext carries a ones column so row 64 of outT accumulates the softmax
denominator. The epilogue broadcasts that row to partitions 0..63 (DRAM
bounce), reciprocals on DVE, multiplies, and DMAs outT back; the host
transposes to [L, D].
"""

import math
from contextlib import ExitStack

import ml_dtypes
import numpy as np

import concourse.bacc as bacc
import concourse.mybir as mybir
import concourse.tile as tile
from concourse.bass_utils import run_bass_kernel_spmd

B, L, H, E, D = 2, 2048, 8, 64, 64
NCORES = 8
PAIRS = (B * H) // NCORES          # (b,h) pairs per core = 2
NATT = 2 * PAIRS                   # attentions per core = 4
SCALE = 1.0 / math.sqrt(E)
P = 128                            # partitions / key-tile size
NKT = L // P                       # 16 key tiles
HALF = L // 2                      # query half-pass size (PSUM budget)
F32 = mybir.dt.float32
BF16 = mybir.dt.bfloat16

_CACHE = {}


def _build_nc():
    nc = bacc.Bacc()
    qt = nc.declare_dram_parameter("qt", [P, 2, L], BF16, isOutput=False)
    kt = nc.declare_dram_parameter("kt", [P, 2, L], BF16, isOutput=False)
    ve = nc.declare_dram_parameter("ve", [P, NATT, NKT, D + 1], BF16, isOutput=False)
    out = nc.declare_dram_parameter("out", [NATT, D, L], F32, isOutput=True)

    with tile.TileContext(nc) as tc:
        with ExitStack() as ctx:
            _body(ctx, tc, qt, kt, ve, out)
    nc.finalize()
    return nc


def _body(ctx, tc, qt, kt, ve, out):
    nc = tc.nc
    Exp = mybir.ActivationFunctionType.Exp

    rec_dram = [
        nc.dram_tensor(f"rec_dram{i}", [1, HALF], F32) for i in range(2 * NATT)
    ]
    persist = ctx.enter_context(tc.tile_pool(name="persist", bufs=1))
    s_psum = ctx.enter_context(tc.tile_pool(name="s_psum", bufs=2, space="PSUM"))
    pv_psum = ctx.enter_context(tc.tile_pool(name="pv_psum", bufs=1, space="PSUM"))
    p_pool = ctx.enter_context(tc.tile_pool(name="p_pool", bufs=3))
    o_pool = ctx.enter_context(tc.tile_pool(name="o_pool", bufs=3))
    small = ctx.enter_context(tc.tile_pool(name="small", bufs=3))

    qt_sb = persist.tile([P, 2, L], BF16)
    kt_sb = persist.tile([P, 2, L], BF16)
    ve_sb = persist.tile([P, NATT, NKT, D + 1], BF16)
    for g in range(PAIRS):
        nc.default_dma_engine.dma_start(out=kt_sb[:, g, :], in_=kt[:, g, :])
        nc.default_dma_engine.dma_start(out=qt_sb[:, g, :], in_=qt[:, g, :])
        for t in range(2):
            a = 2 * g + t
            nc.default_dma_engine.dma_start(
                out=ve_sb[:, a, :, :], in_=ve[:, a, :, :]
            )

    for g in range(PAIRS):
        for pss in range(2):
            q0, q1 = pss * HALF, (pss + 1) * HALF
            kts = [k for k in range(NKT) if P * k < q1]
            pv_ps = [
                pv_psum.tile([D + 1, HALF], F32, tag=f"pv{t}", name=f"pv{t}")
                for t in range(2)
            ]
            def emit_pv(k, ki, pTk):
                # PV accumulate; chunks aligned to the psum 512 bank grid
                qlo = max(q0, P * k)
                off = qlo - q0
                for t in range(2):
                    a = 2 * g + t
                    b0 = off
                    while b0 < HALF:
                        b1 = min((b0 // 512 + 1) * 512, HALF)
                        nc.tensor.matmul(
                            pv_ps[t][:, b0:b1],
                            lhsT=ve_sb[:, a, k, :],
                            rhs=pTk[t][:, b0 - off:b1 - off],
                            start=(ki == 0),
                            stop=(k == kts[-1]),
                            skip_group_check=True,
                        )
                        b0 = b1

            # software-pipelined by one key tile: tile ki's PV matmuls are
            # emitted after tile ki+1's score matmuls so the PE never waits
            # on the just-issued exp
            pend = None
            for ki, k in enumerate(kts):
                qlo = max(q0, P * k)
                w = q1 - qlo
                diag = qlo == P * k
                # score matmuls for the two attn types interleaved (PE row
                # groups 0-63 / 64-127); each 512-chunk gets its own 1-bank
                # psum tile so exp frees score slots at 512 granularity and
                # the next tile's scores double-buffer against this exp
                chunks = {0: [], 1: []}
                for c0 in range(0, w, 512):
                    c1 = min(c0 + 512, w)
                    for t in range(2):
                        bp = 64 * t
                        s_c = s_psum.tile([P, 512], F32, tag=f"s{t}", name=f"s{t}")
                        nc.tensor.matmul(
                            s_c[:, 0:c1 - c0],
                            lhsT=kt_sb[bp:bp + 64, g, P * k:P * (k + 1)],
                            rhs=qt_sb[bp:bp + 64, g, qlo + c0:qlo + c1],
                            start=True,
                            stop=True,
                            skip_group_check=True,
                        )
                        chunks[t].append((s_c, c0, c1))
                if pend is not None:
                    emit_pv(*pend)
                cur_pT = []
                for t in range(2):
                    pT = p_pool.tile([P, HALF], BF16, tag=f"p{t}", name=f"p{t}")
                    for s_c, c0, c1 in chunks[t]:
                        nc.scalar.activation(
                            pT[:, c0:c1], s_c[:, 0:c1 - c0], Exp, scale=SCALE
                        )
                    if diag:
                        # diagonal block: zero where q < key (iota = j - part)
                        nc.gpsimd.affine_select(
                            out=pT[:, 0:P],
                            in_=pT[:, 0:P],
                            compare_op=mybir.AluOpType.is_ge,
                            fill=0.0,
                            base=0,
                            channel_multiplier=-1,
                            pattern=[[1, P]],
                        )
                    cur_pT.append(pT)
                pend = (k, ki, cur_pT)
            emit_pv(*pend)
            for t in range(2):
                a = 2 * g + t
                o_sb = o_pool.tile([D + 1, HALF], F32, tag="o")
                nc.vector.tensor_copy(out=o_sb, in_=pv_ps[t])
                # broadcast the denominator row to partitions 0..63 via a
                # DRAM bounce, then reciprocal at base partition 0
                rb_dram = rec_dram[2 * (2 * g + pss) + t]
                nc.default_dma_engine.dma_start(out=rb_dram[:, :], in_=o_sb[D:D + 1, :])
                den_b = small.tile([D, HALF], F32, tag="den_b")
                nc.default_dma_engine.dma_start(
                    out=den_b, in_=rb_dram[:, :].to_broadcast([D, HALF])
                )
                rec_b = small.tile([D, HALF], F32, tag="rec_b")
                scr_b = small.tile([D, HALF], F32, tag="scr_b")
                nc.vector.reciprocal_approx_accurate(
                    out=rec_b, in_=den_b, scratch=scr_b
                )
                o_n = o_pool.tile([D, HALF], F32, tag="o_n")
                nc.vector.tensor_mul(o_n, o_sb[0:D, :], rec_b)
                nc.default_dma_engine.dma_start(
                    out=out[a, :, q0:q1], in_=o_n
                )


def _host_shard(inputs):
    """Build the 8 per-core input maps from full inputs (host-side numpy)."""
    q_t = np.asarray(inputs["queries_time"], dtype=np.float32)
    k_t = np.asarray(inputs["keys_time"], dtype=np.float32)
    v_t = np.asarray(inputs["values_time"], dtype=np.float32)
    q_c = np.asarray(inputs["queries_channel"], dtype=np.float32)
    k_c = np.asarray(inputs["keys_channel"], dtype=np.float32)
    v_c = np.asarray(inputs["values_channel"], dtype=np.float32)

    bf16 = ml_dtypes.bfloat16
    in_maps = []
    for c in range(NCORES):
        qt = np.empty((P, 2, L), np.float32)
        kt = np.empty((P, 2, L), np.float32)
        ve = np.empty((P, NATT, NKT, D + 1), np.float32)
        for g in range(PAIRS):
            p = PAIRS * c + g
            b, h = divmod(p, H)
            qt[:64, g, :] = q_t[b, :, h, :].T
            qt[64:, g, :] = q_c[b, :, h, :].T
            kt[:64, g, :] = k_t[b, :, h, :].T
            kt[64:, g, :] = k_c[b, :, h, :].T
            for t, v_full in enumerate((v_t, v_c)):
                a = 2 * g + t
                # ve[p_row, a, ktile, 0:64] = V[ktile*128 + p_row, :]
                ve[:, a, :, :D] = v_full[b, :, h, :].reshape(NKT, P, D).transpose(1, 0, 2)
                ve[:, a, :, D] = 1.0
        in_maps.append({
            "qt": np.ascontiguousarray(qt).astype(bf16),
            "kt": np.ascontiguousarray(kt).astype(bf16),
            "ve": np.ascontiguousarray(ve).astype(bf16),
        })
    return in_maps


def _run(in_maps, trace=False):
    if "nc" not in _CACHE:
        _CACHE["nc"] = _build_nc()
    return run_bass_kernel_spmd(
        _CACHE["nc"], in_maps, core_ids=list(range(NCORES)), trace=trace
    )


def kernel(**inputs):
    in_maps = _host_shard(inputs)
    res = _run(in_maps, trace=False)
    v_time = np.empty((B, L, H, D), np.float32)
    v_chan = np.empty((B, L, H, D), np.float32)
    for c in range(NCORES):
        o = np.asarray(res.results[c]["out"])  # [NATT, D, L]
        for g in range(PAIRS):
            p = PAIRS * c + g
            b, h = divmod(p, H)
            v_time[b, :, h, :] = o[2 * g + 0].T
            v_chan[b, :, h, :] = o[2 * g + 1].T
    return v_time, v_chan


# revision 37
# speedup vs baseline: 1.1445x; 1.1445x over previous
"""AnomalyAttention (two causal attentions per (b,h)) on 8 TRN2 NeuronCores.

Sharding: B*H = 16 (batch, head) pairs -> 2 pairs per core. Each core runs
4 independent causal attentions (time + channel for each of its 2 pairs).
No cross-core communication.

Per-attention layout ("transposed flash"): keys live on SBUF partitions.
  S^T[k, q]   = kT_tile.T @ qT                (PE bf16, contraction E=64;
                the two attn types use PE row-groups 0-63 / 64-127)
  P^T         = exp(scale * S^T)              (ACT, PSUM -> SBUF bf16)
  diag mask   = affine_select zero-triangle   (GPSIMD, on the diag block)
  outT[d, q] += V_ext[k, d].T @ P^T[k, q]     (PE bf16, PSUM-accumulated)
V_ext carries a ones column so row 64 of outT accumulates the softmax
denominator. The epilogue broadcasts that row to partitions 0..63 (DRAM
bounce), reciprocals on DVE, multiplies, and DMAs outT back; the host
transposes to [L, D].
"""

import math
from contextlib import ExitStack

import ml_dtypes
import numpy as np

import concourse.bacc as bacc
import concourse.mybir as mybir
import concourse.tile as tile
from concourse.bass_utils import run_bass_kernel_spmd

B, L, H, E, D = 2, 2048, 8, 64, 64
NCORES = 8
PAIRS = (B * H) // NCORES          # (b,h) pairs per core = 2
NATT = 2 * PAIRS                   # attentions per core = 4
SCALE = 1.0 / math.sqrt(E)
P = 128                            # partitions / key-tile size
NKT = L // P                       # 16 key tiles
HALF = L // 2                      # query half-pass size (PSUM budget)
F32 = mybir.dt.float32
BF16 = mybir.dt.bfloat16

_CACHE = {}


def _build_nc():
    nc = bacc.Bacc()
    qt = nc.declare_dram_parameter("qt", [P, 2, L], BF16, isOutput=False)
    kt = nc.declare_dram_parameter("kt", [P, 2, L], BF16, isOutput=False)
    ve = nc.declare_dram_parameter("ve", [P, NATT, NKT, D + 1], BF16, isOutput=False)
    out = nc.declare_dram_parameter("out", [NATT, D, L], F32, isOutput=True)

    with tile.TileContext(nc) as tc:
        with ExitStack() as ctx:
            _body(ctx, tc, qt, kt, ve, out)
    nc.finalize()
    return nc


def _body(ctx, tc, qt, kt, ve, out):
    nc = tc.nc
    Exp = mybir.ActivationFunctionType.Exp

    rec_dram = [
        nc.dram_tensor(f"rec_dram{i}", [1, HALF], F32) for i in range(2 * NATT)
    ]
    persist = ctx.enter_context(tc.tile_pool(name="persist", bufs=1))
    s_psum = ctx.enter_context(tc.tile_pool(name="s_psum", bufs=1, space="PSUM"))
    pv_psum = ctx.enter_context(tc.tile_pool(name="pv_psum", bufs=1, space="PSUM"))
    p_pool = ctx.enter_context(tc.tile_pool(name="p_pool", bufs=3))
    o_pool = ctx.enter_context(tc.tile_pool(name="o_pool", bufs=3))
    small = ctx.enter_context(tc.tile_pool(name="small", bufs=3))

    qt_sb = persist.tile([P, 2, L], BF16)
    kt_sb = persist.tile([P, 2, L], BF16)
    ve_sb = persist.tile([P, NATT, NKT, D + 1], BF16)
    for g in range(PAIRS):
        nc.default_dma_engine.dma_start(out=kt_sb[:, g, :], in_=kt[:, g, :])
        nc.default_dma_engine.dma_start(out=qt_sb[:, g, :], in_=qt[:, g, :])
        for t in range(2):
            a = 2 * g + t
            nc.default_dma_engine.dma_start(
                out=ve_sb[:, a, :, :], in_=ve[:, a, :, :]
            )

    for g in range(PAIRS):
        for pss in range(2):
            q0, q1 = pss * HALF, (pss + 1) * HALF
            kts = [k for k in range(NKT) if P * k < q1]
            pv_ps = [
                pv_psum.tile([D + 1, HALF], F32, tag=f"pv{t}", name=f"pv{t}")
                for t in range(2)
            ]
            def emit_pv(k, ki, pTk):
                # PV accumulate; chunks aligned to the psum 512 bank grid
                qlo = max(q0, P * k)
                off = qlo - q0
                for t in range(2):
                    a = 2 * g + t
                    b0 = off
                    while b0 < HALF:
                        b1 = min((b0 // 512 + 1) * 512, HALF)
                        nc.tensor.matmul(
                            pv_ps[t][:, b0:b1],
                            lhsT=ve_sb[:, a, k, :],
                            rhs=pTk[t][:, b0 - off:b1 - off],
                            start=(ki == 0),
                            stop=(k == kts[-1]),
                            skip_group_check=True,
                        )
                        b0 = b1

            # software-pipelined by one key tile: tile ki's PV matmuls are
            # emitted after tile ki+1's score matmuls so the PE never waits
            # on the just-issued exp
            pend = None
            for ki, k in enumerate(kts):
                qlo = max(q0, P * k)
                w = q1 - qlo
                diag = qlo == P * k
                # score matmuls for the two attn types interleaved: they use
                # PE row groups 0-63 / 64-127, so adjacent issue lets the
                # array work on both concurrently
                s_t = [
                    s_psum.tile([P, HALF], F32, tag=f"s{t}", name=f"s{t}")
                    for t in range(2)
                ]
                for c0 in range(0, w, 512):
                    c1 = min(c0 + 512, w)
                    for t in range(2):
                        bp = 64 * t
                        nc.tensor.matmul(
                            s_t[t][:, c0:c1],
                            lhsT=kt_sb[bp:bp + 64, g, P * k:P * (k + 1)],
                            rhs=qt_sb[bp:bp + 64, g, qlo + c0:qlo + c1],
                            start=True,
                            stop=True,
                            skip_group_check=True,
                        )
                if pend is not None:
                    emit_pv(*pend)
                cur_pT = []
                for t in range(2):
                    pT = p_pool.tile([P, HALF], BF16, tag=f"p{t}", name=f"p{t}")
                    nc.scalar.activation(pT[:, :w], s_t[t][:, :w], Exp, scale=SCALE)
                    if diag:
                        # diagonal block: zero where q < key (iota = j - part)
                        nc.gpsimd.affine_select(
                            out=pT[:, 0:P],
                            in_=pT[:, 0:P],
                            compare_op=mybir.AluOpType.is_ge,
                            fill=0.0,
                            base=0,
                            channel_multiplier=-1,
                            pattern=[[1, P]],
                        )
                    cur_pT.append(pT)
                pend = (k, ki, cur_pT)
            emit_pv(*pend)
            for t in range(2):
                a = 2 * g + t
                o_sb = o_pool.tile([D + 1, HALF], F32, tag="o")
                nc.vector.tensor_copy(out=o_sb, in_=pv_ps[t])
                # broadcast the denominator row to partitions 0..63 via a
                # DRAM bounce, then reciprocal at base partition 0
                rb_dram = rec_dram[2 * (2 * g + pss) + t]
                nc.default_dma_engine.dma_start(out=rb_dram[:, :], in_=o_sb[D:D + 1, :])
                den_b = small.tile([D, HALF], F32, tag="den_b")
                nc.default_dma_engine.dma_start(
                    out=den_b, in_=rb_dram[:, :].to_broadcast([D, HALF])
                )
                rec_b = small.tile([D, HALF], F32, tag="rec_b")
                scr_b = small.tile([D, HALF], F32, tag="scr_b")
                nc.vector.reciprocal_approx_accurate(
                    out=rec_b, in_=den_b, scratch=scr_b
                )
                o_n = o_pool.tile([D, HALF], F32, tag="o_n")
                nc.vector.tensor_mul(o_n, o_sb[0:D, :], rec_b)
                nc.default_dma_engine.dma_start(
                    out=out[a, :, q0:q1], in_=o_n
                )


def _host_shard(inputs):
    """Build the 8 per-core input maps from full inputs (host-side numpy)."""
    q_t = np.asarray(inputs["queries_time"], dtype=np.float32)
    k_t = np.asarray(inputs["keys_time"], dtype=np.float32)
    v_t = np.asarray(inputs["values_time"], dtype=np.float32)
    q_c = np.asarray(inputs["queries_channel"], dtype=np.float32)
    k_c = np.asarray(inputs["keys_channel"], dtype=np.float32)
    v_c = np.asarray(inputs["values_channel"], dtype=np.float32)

    bf16 = ml_dtypes.bfloat16
    in_maps = []
    for c in range(NCORES):
        qt = np.empty((P, 2, L), np.float32)
        kt = np.empty((P, 2, L), np.float32)
        ve = np.empty((P, NATT, NKT, D + 1), np.float32)
        for g in range(PAIRS):
            p = PAIRS * c + g
            b, h = divmod(p, H)
            qt[:64, g, :] = q_t[b, :, h, :].T
            qt[64:, g, :] = q_c[b, :, h, :].T
            kt[:64, g, :] = k_t[b, :, h, :].T
            kt[64:, g, :] = k_c[b, :, h, :].T
            for t, v_full in enumerate((v_t, v_c)):
                a = 2 * g + t
                # ve[p_row, a, ktile, 0:64] = V[ktile*128 + p_row, :]
                ve[:, a, :, :D] = v_full[b, :, h, :].reshape(NKT, P, D).transpose(1, 0, 2)
                ve[:, a, :, D] = 1.0
        in_maps.append({
            "qt": np.ascontiguousarray(qt).astype(bf16),
            "kt": np.ascontiguousarray(kt).astype(bf16),
            "ve": np.ascontiguousarray(ve).astype(bf16),
        })
    return in_maps


def _run(in_maps, trace=False):
    if "nc" not in _CACHE:
        _CACHE["nc"] = _build_nc()
    return run_bass_kernel_spmd(
        _CACHE["nc"], in_maps, core_ids=list(range(NCORES)), trace=trace
    )


def kernel(**inputs):
    in_maps = _host_shard(inputs)
    res = _run(in_maps, trace=False)
    v_time = np.empty((B, L, H, D), np.float32)
    v_chan = np.empty((B, L, H, D), np.float32)
    for c in range(NCORES):
        o = np.asarray(res.results[c]["out"])  # [NATT, D, L]
        for g in range(PAIRS):
            p = PAIRS * c + g
            b, h = divmod(p, H)
            v_time[b, :, h, :] = o[2 * g + 0].T
            v_chan[b, :, h, :] = o[2 * g + 1].T
    return v_time, v_chan
